# revision 10
# baseline (speedup 1.0000x reference)
"""Additive (Bahdanau) attention on 8 Trainium2 NeuronCores.

Problem shapes (hardcoded): B=16, Q=64, K=512, DQ=DK=DV=512, H=256.

Strategy
--------
The 16x64 = 1024 (batch, query) rows are split into 64 units of
(batch, 16 queries).  The graph is specialized at build time on
valid_lens: units are sorted by their batch's valid_len and grouped into
8 "slots" of 8 units (one unit per core per slot), so slot s's key
extent EXT_s hugs the sorted valid_len distribution.  Every core runs
the same instruction stream; which unit a (core, slot) pair processes is
pure input data.  Masked key tails beyond EXT_s are never computed
(sparse attention); keys in [valid_len, EXT_s) get an additive -30
folded into the score accumulation as a rank-1 matmul.

Per (slot, q) on device:
  qb_T[h,q] = Wq^T queries_T            (TensorE, once for all slots)
  kb_T[h,k] = Wk^T keys_T               (TensorE, per slot)
  t[h,k]    = kb_T[h,k] + qb_T[h,q]     (VectorE/GpSimd per-partition add)
  t         = tanh(t)                   (ScalarE — the roofline engine)
  scores[q,:] += Wv_win^T t             (TensorE: [128,16] zero-padded
                                         sliding window with Wv in column q
                                         accumulates row q of a PSUM [16,EXT]
                                         scores matrix directly)
  scores    += -30 * maskrow            (rank-1 matmul)
  E = exp(scores), S = rowsum           (ScalarE from PSUM, accum_out)
  attn_T    = E^T                       (TensorE transpose)
  out[q,:]  = (attn_T^T values) / S     (TensorE + VectorE drain with 1/S)

Slots are processed largest-extent first with the tanh split into
chunks, so the ScalarE pipeline fills fast and drains onto the smallest
slot.  All device tensors are float16 (fp32 PSUM accumulation);
host-side prep is layout only (transpose/cast/slice/pad + mask rows).
"""

import numpy as np

import concourse.bass as bass
import concourse.tile as tile
from concourse import mybir
from concourse.bass_utils import run_bass_kernel_spmd
from concourse.masks import make_identity

F16 = mybir.dt.float16
F32 = mybir.dt.float32

B, Q, K, D, H = 16, 64, 512, 512, 256
N_CORES = 8
QG = 16                       # queries per unit
N_SLOTS = (B * Q) // (N_CORES * QG)   # 8
MASK_ADD = -30.0              # exp(-30) ~ 1e-13: numerically zero


def _ceil_to(x, m):
    return ((x + m - 1) // m) * m


def _split_multi_waits(nc):
    """Workaround: this walrus build accepts only ONE sync wait per
    instruction.  Hoist all but the last wait onto preceding same-engine
    InstEventSemaphore instructions (what wait_ge lowers to)."""
    n = 0
    for fn in nc.m.functions:
        for blk in fn.blocks:
            out = []
            for ins in blk.instructions:
                si = getattr(ins, "sync_info", None)
                if si is not None and si.on_wait and len(si.on_wait) > 1:
                    waits = list(si.on_wait)
                    for w in waits[:-1]:
                        ev = mybir.InstEventSemaphore(
                            name=f"waitfix-{n}", ins=[], outs=[])
                        n += 1
                        ev.engine = ins.engine
                        ev.sync_info = mybir.SyncInfo(on_wait=[w], on_update=[])
                        out.append(ev)
                    si.on_wait = [waits[-1]]
                out.append(ins)
            blk.instructions = out
    return n


def build_nc(exts):
    """Build the shared SPMD graph.  exts[s] = key extent of slot s
    (slot 0 largest, descending, all even)."""
    extcs = [_ceil_to(e, 128) for e in exts]
    nc = bass.Bass("TRN2")

    wq_d = nc.declare_dram_parameter("wq", [D, H], F16, isOutput=False)
    wk_d = nc.declare_dram_parameter("wk", [D, H], F16, isOutput=False)
    wv2_d = nc.declare_dram_parameter("wv2", [128, 2 * (2 * QG - 1)], F16,
                                      isOutput=False)
    qt_d = nc.declare_dram_parameter("qt", [D, N_SLOTS * QG], F16,
                                     isOutput=False)
    kt_d = [nc.declare_dram_parameter(f"kt{s}", [D, exts[s]], F16,
                                      isOutput=False) for s in range(N_SLOTS)]
    v_d = [nc.declare_dram_parameter(f"v{s}", [extcs[s], 512], F16,
                                     isOutput=False) for s in range(N_SLOTS)]
    m_d = [nc.declare_dram_parameter(f"mask{s}", [1, exts[s]], F16,
                                     isOutput=False) for s in range(N_SLOTS)]
    out_d = nc.declare_dram_parameter("out", [N_SLOTS, QG, 512], F32,
                                      isOutput=True)

    with tile.TileContext(nc) as tc, \
            tc.tile_pool(name="consts", bufs=1) as consts, \
            tc.tile_pool(name="kt", bufs=3) as ktp, \
            tc.tile_pool(name="vv", bufs=3) as vvp, \
            tc.tile_pool(name="kb", bufs=3) as kbp, \
            tc.tile_pool(name="t0", bufs=4) as t0p, \
            tc.tile_pool(name="sm", bufs=3) as smp, \
            tc.tile_pool(name="outp", bufs=2) as outp, \
            tc.tile_pool(name="ps_kb", bufs=2, space="PSUM") as ps_kb, \
            tc.tile_pool(name="ps_sc", bufs=3, space="PSUM") as ps_sc, \
            tc.tile_pool(name="ps_et", bufs=1, space="PSUM") as ps_et, \
            tc.tile_pool(name="ps_o", bufs=2, space="PSUM") as ps_o:

        # Warm the ACT table set (tanh/exp share one set) during DMA ramp.
        dummy = consts.tile([1, 2], F16, tag="dummy")
        nc.vector.memset(dummy, 0.0)
        nc.scalar.activation(dummy[:], dummy[:], mybir.ActivationFunctionType.Tanh)

        # --- constants.  DMA emission order is the ramp-critical path:
        # wk + kt0 feed the first kb projection, wq + qt feed qb.
        wk_sb = consts.tile([128, 4, H], F16, tag="wk")
        for dt in range(4):
            nc.sync.dma_start(
                out=wk_sb[:, dt, :], in_=wk_d[dt * 128:(dt + 1) * 128, :])
        kt_sb = [[ktp.tile([128, exts[s]], F16, tag=f"kt{dt}",
                           name=f"kt_sb{s}_{dt}")
                  for dt in range(4)] for s in range(N_SLOTS)]
        for dt in range(4):
            nc.sync.dma_start(
                out=kt_sb[0][dt], in_=kt_d[0][dt * 128:(dt + 1) * 128, :])
        wq_sb = consts.tile([128, 4, H], F16, tag="wq")
        for dt in range(4):
            nc.sync.dma_start(
                out=wq_sb[:, dt, :], in_=wq_d[dt * 128:(dt + 1) * 128, :])
        qt_sb = consts.tile([128, 4, N_SLOTS * QG], F16, tag="qt")
        for dt in range(4):
            nc.sync.dma_start(
                out=qt_sb[:, dt, :], in_=qt_d[dt * 128:(dt + 1) * 128, :])
        for s in range(1, N_SLOTS):
            for dt in range(4):
                nc.sync.dma_start(
                    out=kt_sb[s][dt], in_=kt_d[s][dt * 128:(dt + 1) * 128, :])
        wv2_sb = consts.tile([128, 2 * (2 * QG - 1)], F16, tag="wv2")
        nc.sync.dma_start(out=wv2_sb, in_=wv2_d[:])
        ident = consts.tile([128, 128], F16, tag="ident")
        make_identity(nc, ident[:])
        ones = consts.tile([1, QG], F16, tag="ones")
        nc.vector.memset(ones, 1.0)

        kb_sb = [None] * N_SLOTS

        def proj(s):
            ext = exts[s]
            kb = kbp.tile([128, 2, ext], F16, tag="kb", name=f"kb{s}")
            for ht in range(2):
                kb_ps = ps_kb.tile([128, 512], F32, tag="kb_ps",
                                   name=f"kb_ps{s}_{ht}")
                for dt in range(4):
                    nc.tensor.matmul(
                        kb_ps[:, :ext],
                        wk_sb[:, dt, ht * 128:(ht + 1) * 128],
                        kt_sb[s][dt][:],
                        start=(dt == 0), stop=(dt == 3),
                    )
                nc.vector.tensor_copy(kb[:, ht, :], kb_ps[:, :ext])
            kb_sb[s] = kb

        proj(0)

        # qb_T for ALL slots at once: [128, 2, 128] fp32
        qb_ps = ps_kb.tile([128, 2, N_SLOTS * QG], F32, tag="kb_ps")
        for ht in range(2):
            for dt in range(4):
                nc.tensor.matmul(
                    qb_ps[:, ht, :],
                    wq_sb[:, dt, ht * 128:(ht + 1) * 128],
                    qt_sb[:, dt, :],
                    start=(dt == 0), stop=(dt == 3),
                )
        qb_sb = consts.tile([128, 2, N_SLOTS * QG], F32, tag="qb")
        nc.vector.tensor_copy(qb_sb[:], qb_ps[:])

        proj(1)

        # values + masks (needed late)
        v_sb, m_sb = [], []
        for s in range(N_SLOTS):
            vt = vvp.tile([128, extcs[s] // 128, 512], F16, tag="v",
                          name=f"v_sb{s}")
            nc.sync.dma_start(
                out=vt, in_=v_d[s][:].rearrange("(t p) v -> p t v", p=128))
            mt = smp.tile([1, exts[s]], F16, tag=f"m{s}", name=f"m_sb{s}")
            nc.sync.dma_start(out=mt, in_=m_d[s][:])
            v_sb.append(vt)
            m_sb.append(mt)

        scores_ps = [None] * N_SLOTS

        def main(s):
            ext = exts[s]
            kb = kb_sb[s]
            scores = ps_sc.tile([QG, 512], F32, tag="scores",
                                name=f"scores{s}")
            scores_ps[s] = scores
            chunks = [4, 4, 4, 4] if s == 0 else [8, 8]
            q0 = 0
            for ci, cn in enumerate(chunks):
                t0 = t0p.tile([128, cn, 2, ext], F16, tag="t0",
                              name=f"t0_{s}_{ci}")
                for ql in range(cn):
                    for ht in range(2):
                        qi = s * QG + q0 + ql
                        eng = nc.gpsimd if (ql * 2 + ht) % 3 == 2 else nc.vector
                        eng.tensor_scalar_add(
                            out=t0[:, ql, ht, :],
                            in0=kb[:, ht, :],
                            scalar1=qb_sb[:, ht, qi:qi + 1],
                        )
                nc.scalar.activation(
                    t0[:], t0[:], mybir.ActivationFunctionType.Tanh)
                for ql in range(cn):
                    for ht in range(2):
                        c0 = ht * (2 * QG - 1) + (QG - 1) - (q0 + ql)
                        nc.tensor.matmul(
                            scores[:, :ext],
                            wv2_sb[:, c0:c0 + QG],
                            t0[:, ql, ht, :],
                            start=(q0 + ql == 0 and ht == 0),
                            stop=False,
                        )
                q0 += cn
            nc.tensor.matmul(scores[:, :ext], ones[:], m_sb[s][:],
                             start=False, stop=True)

        def epilogue(s):
            ext, extc = exts[s], extcs[s]
            scores = scores_ps[s]
            e_sb = smp.tile([QG, extc], F16, tag="e", name=f"e{s}")
            ssum = smp.tile([QG, 1], F32, tag="ssum", name=f"ssum{s}")
            sinv = smp.tile([QG, 1], F32, tag="sinv", name=f"sinv{s}")
            if extc > ext:
                nc.vector.memset(e_sb[:, ext:], 0.0)
            nc.scalar.activation(
                e_sb[:, :ext], scores[:, :ext],
                mybir.ActivationFunctionType.Exp, accum_out=ssum[:])
            nc.vector.reciprocal(sinv[:], ssum[:])
            et = smp.tile([128, extc // 128, QG], F16, tag="et", name=f"et{s}")
            for kt_i in range(extc // 128):
                et_ps = ps_et.tile([128, QG], F16, tag="et_ps",
                                   name=f"et_ps{s}_{kt_i}")
                nc.tensor.transpose(
                    et_ps[:], e_sb[:, kt_i * 128:(kt_i + 1) * 128],
                    ident[:QG, :QG])
                nc.vector.tensor_copy(et[:, kt_i, :], et_ps[:])
            o_ps = ps_o.tile([QG, 512], F32, tag="o_ps", name=f"o_ps{s}")
            for kt_i in range(extc // 128):
                nc.tensor.matmul(
                    o_ps[:], et[:, kt_i, :], v_sb[s][:, kt_i, :],
                    start=(kt_i == 0), stop=(kt_i == extc // 128 - 1),
                )
            o_sb = outp.tile([QG, 512], F32, tag="o_sb", name=f"o_sb{s}")
            nc.vector.tensor_scalar_mul(out=o_sb[:], in0=o_ps[:],
                                        scalar1=sinv[:])
            nc.sync.dma_start(out=out_d[s], in_=o_sb[:])

        # pipeline: PE projections stay 2 slots ahead of the main loop
        for s in range(N_SLOTS):
            if s + 2 < N_SLOTS:
                proj(s + 2)
            main(s)
            if s >= 1:
                epilogue(s - 1)
        epilogue(N_SLOTS - 1)

    _split_multi_waits(nc)
    return nc


def _prep(inputs):
    """Shard + lay out inputs; returns (nc, in_maps, assignment)."""
    queries = np.asarray(inputs["queries"], np.float32)
    keys = np.asarray(inputs["keys"], np.float32)
    values = np.asarray(inputs["values"], np.float32)
    vlens = np.asarray(inputs["valid_lens"]).astype(np.int64)
    Wq = np.asarray(inputs["Wq"], np.float32)
    Wk = np.asarray(inputs["Wk"], np.float32)
    Wv = np.asarray(inputs["Wv"], np.float32)

    # units: (batch, q-quarter) sorted by batch valid_len descending;
    # slot s (largest first) <- ranks [8s, 8s+8)
    border = np.argsort(-vlens, kind="stable")
    units = [(int(b), qq) for b in border for qq in range(4)]
    assignment = [[None] * N_SLOTS for _ in range(N_CORES)]
    exts = [0] * N_SLOTS
    for s in range(N_SLOTS):
        group = units[N_CORES * s:N_CORES * (s + 1)]
        exts[s] = _ceil_to(max(int(vlens[b]) for b, _ in group), 2)
        for c in range(N_CORES):
            assignment[c][s] = group[c]
    extcs = [_ceil_to(e, 128) for e in exts]

    wq16 = np.ascontiguousarray(Wq, dtype=np.float16)
    wk16 = np.ascontiguousarray(Wk, dtype=np.float16)
    wv2 = np.zeros((128, 2 * (2 * QG - 1)), np.float16)
    wv2[:, QG - 1] = Wv[:128].astype(np.float16)
    wv2[:, (2 * QG - 1) + QG - 1] = Wv[128:].astype(np.float16)

    keys16 = keys.astype(np.float16)
    queries16 = queries.astype(np.float16)
    values16 = values.astype(np.float16)

    in_maps = []
    for c in range(N_CORES):
        m = {"wq": wq16, "wk": wk16, "wv2": wv2}
        qt = np.empty((D, N_SLOTS * QG), np.float16)
        for s in range(N_SLOTS):
            b, qq = assignment[c][s]
            lb = int(vlens[b])
            qt[:, s * QG:(s + 1) * QG] = queries16[b, qq * QG:(qq + 1) * QG].T
            m[f"kt{s}"] = np.ascontiguousarray(keys16[b, :exts[s]].T)
            v = np.zeros((extcs[s], 512), np.float16)
            v[:min(exts[s], lb)] = values16[b, :min(exts[s], lb)]
            m[f"v{s}"] = v
            mask = np.zeros((1, exts[s]), np.float16)
            mask[0, lb:] = MASK_ADD
            m[f"mask{s}"] = mask
        m["qt"] = qt
        in_maps.append(m)

    nc = build_nc(exts)
    return nc, in_maps, assignment


def _run(inputs, trace=False):
    nc, in_maps, assignment = _prep(inputs)
    res = run_bass_kernel_spmd(
        nc, in_maps, core_ids=list(range(N_CORES)), trace=trace)
    out = np.empty((B, Q, 512), np.float32)
    for c in range(N_CORES):
        o = np.asarray(res.results[c]["out"], np.float32)
        for s in range(N_SLOTS):
            b, qq = assignment[c][s]
            out[b, qq * QG:(qq + 1) * QG] = o[s]
    return out, res


def kernel(**inputs):
    out, _ = _run(inputs, trace=False)
    return out


if __name__ == "__main__":
    rng = np.random.default_rng(0)
    demo = {
        "queries": rng.standard_normal((B, Q, D), dtype=np.float32),
        "keys": rng.standard_normal((B, K, D), dtype=np.float32),
        "values": rng.standard_normal((B, K, D), dtype=np.float32),
        "valid_lens": rng.integers(1, K + 1, size=(B,)).astype(np.int32),
        "Wq": rng.standard_normal((D, H), dtype=np.float32) / np.sqrt(D),
        "Wk": rng.standard_normal((D, H), dtype=np.float32) / np.sqrt(D),
        "Wv": rng.standard_normal((H,), dtype=np.float32) / np.sqrt(H),
    }
    print(kernel(**demo).shape)


# revision 11
# speedup vs baseline: 3.5800x; 3.5800x over previous
"""Additive (Bahdanau) attention on 8 Trainium2 NeuronCores.

Problem shapes (hardcoded): B=16, Q=64, K=512, DQ=DK=DV=512, H=256.

Strategy
--------
The 16x64 = 1024 (batch, query) rows are split into 64 units of
(batch, 16 queries).  The graph is specialized at build time on
valid_lens: units are sorted by their batch's valid_len and grouped into
8 "slots" of 8 units (one unit per core per slot), so slot s's key
extent EXT_s hugs the sorted valid_len distribution.  Every core runs
the same instruction stream; which unit a (core, slot) pair processes is
pure input data.  Masked key tails beyond EXT_s are never computed
(sparse attention); keys in [valid_len, EXT_s) get an additive -30
folded into the score accumulation as a rank-1 matmul.

Per (slot, q) on device:
  qb_T[h,q] = Wq^T queries_T            (TensorE, once for all slots)
  kb_T[h,k] = Wk^T keys_T               (TensorE, per slot)
  t[h,k]    = kb_T[h,k] + qb_T[h,q]     (VectorE/GpSimd per-partition add)
  t         = tanh(t)                   (ScalarE — the roofline engine)
  scores[q,:] += Wv_win^T t             (TensorE: [128,16] zero-padded
                                         sliding window with Wv in column q
                                         accumulates row q of a PSUM [16,EXT]
                                         scores matrix directly)
  scores    += -30 * maskrow            (rank-1 matmul)
  E = exp(scores), S = rowsum           (ScalarE from PSUM, accum_out)
  attn_T    = E^T                       (TensorE transpose)
  out[q,:]  = (attn_T^T values) / S     (TensorE + VectorE drain with 1/S)

Slots are processed largest-extent first with the tanh split into
chunks, so the ScalarE pipeline fills fast and drains onto the smallest
slot.  All device tensors are float16 (fp32 PSUM accumulation);
host-side prep is layout only (transpose/cast/slice/pad + mask rows).
"""

import numpy as np

import concourse.bass as bass
import concourse.tile as tile
from concourse import mybir
from concourse.bass_utils import run_bass_kernel_spmd
from concourse.masks import make_identity

F16 = mybir.dt.float16
F32 = mybir.dt.float32

B, Q, K, D, H = 16, 64, 512, 512, 256
N_CORES = 8
QG = 16                       # queries per unit
N_SLOTS = (B * Q) // (N_CORES * QG)   # 8
MASK_ADD = -30.0              # exp(-30) ~ 1e-13: numerically zero


def _ceil_to(x, m):
    return ((x + m - 1) // m) * m


def _split_multi_waits(nc):
    """Workaround: this walrus build accepts only ONE sync wait per
    instruction.  Hoist all but the last wait onto preceding same-engine
    InstEventSemaphore instructions (what wait_ge lowers to)."""
    n = 0
    for fn in nc.m.functions:
        for blk in fn.blocks:
            out = []
            for ins in blk.instructions:
                si = getattr(ins, "sync_info", None)
                if si is not None and si.on_wait and len(si.on_wait) > 1:
                    waits = list(si.on_wait)
                    for w in waits[:-1]:
                        ev = mybir.InstEventSemaphore(
                            name=f"waitfix-{n}", ins=[], outs=[])
                        n += 1
                        ev.engine = ins.engine
                        ev.sync_info = mybir.SyncInfo(on_wait=[w], on_update=[])
                        out.append(ev)
                    si.on_wait = [waits[-1]]
                out.append(ins)
            blk.instructions = out
    return n


def build_nc(exts):
    """Build the shared SPMD graph.  exts[s] = key extent of slot s
    (slot 0 largest, descending, all even)."""
    extcs = [_ceil_to(e, 128) for e in exts]
    nc = bass.Bass("TRN2")

    wq_d = nc.declare_dram_parameter("wq", [D, H], F16, isOutput=False)
    wk_d = nc.declare_dram_parameter("wk", [D, H], F16, isOutput=False)
    wv2_d = nc.declare_dram_parameter("wv2", [128, 2 * (2 * QG - 1)], F16,
                                      isOutput=False)
    qt_d = nc.declare_dram_parameter("qt", [D, N_SLOTS * QG], F16,
                                     isOutput=False)
    kt_d = [nc.declare_dram_parameter(f"kt{s}", [D, exts[s]], F16,
                                      isOutput=False) for s in range(N_SLOTS)]
    v_d = [nc.declare_dram_parameter(f"v{s}", [extcs[s], 512], F16,
                                     isOutput=False) for s in range(N_SLOTS)]
    m_d = [nc.declare_dram_parameter(f"mask{s}", [1, exts[s]], F16,
                                     isOutput=False) for s in range(N_SLOTS)]
    out_d = nc.declare_dram_parameter("out", [N_SLOTS, QG, 512], F32,
                                      isOutput=True)

    with tile.TileContext(nc) as tc, \
            tc.tile_pool(name="consts", bufs=1) as consts, \
            tc.tile_pool(name="kt", bufs=3) as ktp, \
            tc.tile_pool(name="vv", bufs=3) as vvp, \
            tc.tile_pool(name="kb", bufs=3) as kbp, \
            tc.tile_pool(name="t0", bufs=4) as t0p, \
            tc.tile_pool(name="sm", bufs=3) as smp, \
            tc.tile_pool(name="outp", bufs=2) as outp, \
            tc.tile_pool(name="ps_kb", bufs=2, space="PSUM") as ps_kb, \
            tc.tile_pool(name="ps_sc", bufs=3, space="PSUM") as ps_sc, \
            tc.tile_pool(name="ps_et", bufs=1, space="PSUM") as ps_et, \
            tc.tile_pool(name="ps_o", bufs=2, space="PSUM") as ps_o:

        # Warm the ACT table set (tanh/exp share one set) during DMA ramp.
        dummy = consts.tile([1, 2], F16, tag="dummy")
        nc.vector.memset(dummy, 0.0)
        nc.scalar.activation(dummy[:], dummy[:], mybir.ActivationFunctionType.Tanh)

        # --- constants.  DMA emission order is the ramp-critical path:
        # wk + kt0 feed the first kb projection, wq + qt feed qb.
        wk_sb = consts.tile([128, 4, H], F16, tag="wk")
        for dt in range(4):
            nc.sync.dma_start(
                out=wk_sb[:, dt, :], in_=wk_d[dt * 128:(dt + 1) * 128, :])
        kt_sb = [[ktp.tile([128, exts[s]], F16, tag=f"kt{dt}",
                           name=f"kt_sb{s}_{dt}")
                  for dt in range(4)] for s in range(N_SLOTS)]
        for dt in range(4):
            nc.sync.dma_start(
                out=kt_sb[0][dt], in_=kt_d[0][dt * 128:(dt + 1) * 128, :])
        wq_sb = consts.tile([128, 4, H], F16, tag="wq")
        for dt in range(4):
            nc.sync.dma_start(
                out=wq_sb[:, dt, :], in_=wq_d[dt * 128:(dt + 1) * 128, :])
        qt_sb = consts.tile([128, 4, N_SLOTS * QG], F16, tag="qt")
        for dt in range(4):
            nc.sync.dma_start(
                out=qt_sb[:, dt, :], in_=qt_d[dt * 128:(dt + 1) * 128, :])
        for s in range(1, N_SLOTS):
            for dt in range(4):
                nc.sync.dma_start(
                    out=kt_sb[s][dt], in_=kt_d[s][dt * 128:(dt + 1) * 128, :])
        wv2_sb = consts.tile([128, 2 * (2 * QG - 1)], F16, tag="wv2")
        nc.sync.dma_start(out=wv2_sb, in_=wv2_d[:])
        ident = consts.tile([128, 128], F16, tag="ident")
        make_identity(nc, ident[:])
        ones = consts.tile([1, QG], F16, tag="ones")
        nc.vector.memset(ones, 1.0)

        kb_sb = [None] * N_SLOTS

        def proj(s):
            ext = exts[s]
            kb = kbp.tile([128, 2, ext], F16, tag="kb", name=f"kb{s}")
            for ht in range(2):
                kb_ps = ps_kb.tile([128, 512], F32, tag="kb_ps",
                                   name=f"kb_ps{s}_{ht}")
                for dt in range(4):
                    nc.tensor.matmul(
                        kb_ps[:, :ext],
                        wk_sb[:, dt, ht * 128:(ht + 1) * 128],
                        kt_sb[s][dt][:],
                        start=(dt == 0), stop=(dt == 3),
                    )
                nc.vector.tensor_copy(kb[:, ht, :], kb_ps[:, :ext])
            kb_sb[s] = kb

        proj(0)

        # qb_T for ALL slots at once: [128, 2, 128] fp32
        qb_ps = ps_kb.tile([128, 2, N_SLOTS * QG], F32, tag="kb_ps")
        for ht in range(2):
            for dt in range(4):
                nc.tensor.matmul(
                    qb_ps[:, ht, :],
                    wq_sb[:, dt, ht * 128:(ht + 1) * 128],
                    qt_sb[:, dt, :],
                    start=(dt == 0), stop=(dt == 3),
                )
        qb_sb = consts.tile([128, 2, N_SLOTS * QG], F32, tag="qb")
        nc.vector.tensor_copy(qb_sb[:], qb_ps[:])

        proj(1)

        # values + masks (needed late)
        v_sb, m_sb = [], []
        for s in range(N_SLOTS):
            vt = vvp.tile([128, extcs[s] // 128, 512], F16, tag="v",
                          name=f"v_sb{s}")
            nc.sync.dma_start(
                out=vt, in_=v_d[s][:].rearrange("(t p) v -> p t v", p=128))
            mt = smp.tile([1, exts[s]], F16, tag=f"m{s}", name=f"m_sb{s}")
            nc.sync.dma_start(out=mt, in_=m_d[s][:])
            v_sb.append(vt)
            m_sb.append(mt)

        scores_ps = [None] * N_SLOTS

        def main(s):
            ext = exts[s]
            kb = kb_sb[s]
            scores = ps_sc.tile([QG, 512], F32, tag="scores",
                                name=f"scores{s}")
            scores_ps[s] = scores
            chunks = [4, 4, 4, 4] if s == 0 else [8, 8]
            q0 = 0
            for ci, cn in enumerate(chunks):
                t0 = t0p.tile([128, cn, 2, ext], F16, tag="t0",
                              name=f"t0_{s}_{ci}")
                for ql in range(cn):
                    for ht in range(2):
                        qi = s * QG + q0 + ql
                        nc.vector.tensor_scalar_add(
                            out=t0[:, ql, ht, :],
                            in0=kb[:, ht, :],
                            scalar1=qb_sb[:, ht, qi:qi + 1],
                        )
                nc.scalar.activation(
                    t0[:], t0[:], mybir.ActivationFunctionType.Tanh)
                for ql in range(cn):
                    for ht in range(2):
                        c0 = ht * (2 * QG - 1) + (QG - 1) - (q0 + ql)
                        nc.tensor.matmul(
                            scores[:, :ext],
                            wv2_sb[:, c0:c0 + QG],
                            t0[:, ql, ht, :],
                            start=(q0 + ql == 0 and ht == 0),
                            stop=False,
                        )
                q0 += cn
            nc.tensor.matmul(scores[:, :ext], ones[:], m_sb[s][:],
                             start=False, stop=True)

        def epilogue(s):
            ext, extc = exts[s], extcs[s]
            scores = scores_ps[s]
            e_sb = smp.tile([QG, extc], F16, tag="e", name=f"e{s}")
            ssum = smp.tile([QG, 1], F32, tag="ssum", name=f"ssum{s}")
            sinv = smp.tile([QG, 1], F32, tag="sinv", name=f"sinv{s}")
            if extc > ext:
                nc.vector.memset(e_sb[:, ext:], 0.0)
            nc.scalar.activation(
                e_sb[:, :ext], scores[:, :ext],
                mybir.ActivationFunctionType.Exp, accum_out=ssum[:])
            nc.vector.reciprocal(sinv[:], ssum[:])
            et = smp.tile([128, extc // 128, QG], F16, tag="et", name=f"et{s}")
            for kt_i in range(extc // 128):
                et_ps = ps_et.tile([128, QG], F16, tag="et_ps",
                                   name=f"et_ps{s}_{kt_i}")
                nc.tensor.transpose(
                    et_ps[:], e_sb[:, kt_i * 128:(kt_i + 1) * 128],
                    ident[:QG, :QG])
                nc.vector.tensor_copy(et[:, kt_i, :], et_ps[:])
            o_ps = ps_o.tile([QG, 512], F32, tag="o_ps", name=f"o_ps{s}")
            for kt_i in range(extc // 128):
                nc.tensor.matmul(
                    o_ps[:], et[:, kt_i, :], v_sb[s][:, kt_i, :],
                    start=(kt_i == 0), stop=(kt_i == extc // 128 - 1),
                )
            o_sb = outp.tile([QG, 512], F32, tag="o_sb", name=f"o_sb{s}")
            nc.vector.tensor_scalar_mul(out=o_sb[:], in0=o_ps[:],
                                        scalar1=sinv[:])
            nc.sync.dma_start(out=out_d[s], in_=o_sb[:])

        # pipeline: PE projections stay 2 slots ahead of the main loop
        for s in range(N_SLOTS):
            if s + 2 < N_SLOTS:
                proj(s + 2)
            main(s)
            if s >= 1:
                epilogue(s - 1)
        epilogue(N_SLOTS - 1)

    _split_multi_waits(nc)
    return nc


def _prep(inputs):
    """Shard + lay out inputs; returns (nc, in_maps, assignment)."""
    queries = np.asarray(inputs["queries"], np.float32)
    keys = np.asarray(inputs["keys"], np.float32)
    values = np.asarray(inputs["values"], np.float32)
    vlens = np.asarray(inputs["valid_lens"]).astype(np.int64)
    Wq = np.asarray(inputs["Wq"], np.float32)
    Wk = np.asarray(inputs["Wk"], np.float32)
    Wv = np.asarray(inputs["Wv"], np.float32)

    # units: (batch, q-quarter) sorted by batch valid_len descending;
    # slot s (largest first) <- ranks [8s, 8s+8)
    border = np.argsort(-vlens, kind="stable")
    units = [(int(b), qq) for b in border for qq in range(4)]
    assignment = [[None] * N_SLOTS for _ in range(N_CORES)]
    exts = [0] * N_SLOTS
    for s in range(N_SLOTS):
        group = units[N_CORES * s:N_CORES * (s + 1)]
        exts[s] = _ceil_to(max(int(vlens[b]) for b, _ in group), 2)
        for c in range(N_CORES):
            assignment[c][s] = group[c]
    extcs = [_ceil_to(e, 128) for e in exts]

    wq16 = np.ascontiguousarray(Wq, dtype=np.float16)
    wk16 = np.ascontiguousarray(Wk, dtype=np.float16)
    wv2 = np.zeros((128, 2 * (2 * QG - 1)), np.float16)
    wv2[:, QG - 1] = Wv[:128].astype(np.float16)
    wv2[:, (2 * QG - 1) + QG - 1] = Wv[128:].astype(np.float16)

    keys16 = keys.astype(np.float16)
    queries16 = queries.astype(np.float16)
    values16 = values.astype(np.float16)

    in_maps = []
    for c in range(N_CORES):
        m = {"wq": wq16, "wk": wk16, "wv2": wv2}
        qt = np.empty((D, N_SLOTS * QG), np.float16)
        for s in range(N_SLOTS):
            b, qq = assignment[c][s]
            lb = int(vlens[b])
            qt[:, s * QG:(s + 1) * QG] = queries16[b, qq * QG:(qq + 1) * QG].T
            m[f"kt{s}"] = np.ascontiguousarray(keys16[b, :exts[s]].T)
            v = np.zeros((extcs[s], 512), np.float16)
            v[:min(exts[s], lb)] = values16[b, :min(exts[s], lb)]
            m[f"v{s}"] = v
            mask = np.zeros((1, exts[s]), np.float16)
            mask[0, lb:] = MASK_ADD
            m[f"mask{s}"] = mask
        m["qt"] = qt
        in_maps.append(m)

    nc = build_nc(exts)
    return nc, in_maps, assignment


def _run(inputs, trace=False):
    nc, in_maps, assignment = _prep(inputs)
    res = run_bass_kernel_spmd(
        nc, in_maps, core_ids=list(range(N_CORES)), trace=trace)
    out = np.empty((B, Q, 512), np.float32)
    for c in range(N_CORES):
        o = np.asarray(res.results[c]["out"], np.float32)
        for s in range(N_SLOTS):
            b, qq = assignment[c][s]
            out[b, qq * QG:(qq + 1) * QG] = o[s]
    return out, res


def kernel(**inputs):
    out, _ = _run(inputs, trace=False)
    return out


if __name__ == "__main__":
    rng = np.random.default_rng(0)
    demo = {
        "queries": rng.standard_normal((B, Q, D), dtype=np.float32),
        "keys": rng.standard_normal((B, K, D), dtype=np.float32),
        "values": rng.standard_normal((B, K, D), dtype=np.float32),
        "valid_lens": rng.integers(1, K + 1, size=(B,)).astype(np.int32),
        "Wq": rng.standard_normal((D, H), dtype=np.float32) / np.sqrt(D),
        "Wk": rng.standard_normal((D, H), dtype=np.float32) / np.sqrt(D),
        "Wv": rng.standard_normal((H,), dtype=np.float32) / np.sqrt(H),
    }
    print(kernel(**demo).shape)
